# revision 21
# baseline (speedup 1.0000x reference)
"""Trainium2 Bass kernel for the BDH-style sparse-attention network.

Reference computation (per batch b, all fp32):
  v = LN(wte[idx])                                   [T, D]
  repeat L times:
    x   = relu(v @ Dx)                               [T, N]   (Dx: [D, N] = decoder_x heads concat)
    a   = causal_linear_attention(v) (RoPE, no softmax, tril mask)
    y   = relu(LN(a) @ Dy) * x                       [T, N]
    v   = v + LN(y @ E)                              [T, D]   (E: [N, D] = encoder)
  logits = v @ readout                               [T, VOCAB]

Sharding over 8 NeuronCores: core c -> batch b = c//2, neuron half h = c%2.
Each core holds half the neuron dim (N/2 columns of Dx/Dy, N/2 rows of E) and
computes the full attention for its batch; the partial `y @ E` update is
summed with an AllReduce over core pairs [[0,1],[2,3],[4,5],[6,7]].

On-device layout: token-major tiles v [128t, D] plus a transposed copy
vT [128d, T] maintained via PE transposes, so every matmul has its
contraction dim on partitions without extra data movement.

Pipeline: v / vT / qT are double-buffered by layer parity so the AllReduce +
LN(update) tail of layer l overlaps layer l+1's first chunks.  The second
half's vnew is deferred into the next layer (emitted after its chunk 0).
RoPE runs on the otherwise-idle Pool engine one chunk ahead; the x-relu runs
on the Activation engine; relu(y)*x is a single fused DVE op reading PSUM.
MLP weights / rope tables are bf16 and SBUF-resident.
"""

import numpy as np

import concourse.bass as bass
import concourse.bacc as bacc
import concourse.mybir as mybir
import concourse.tile as tile
from concourse.bass_utils import run_bass_kernel_spmd

FP = mybir.dt.float32
AX = mybir.AxisListType
ALU = mybir.AluOpType
ACTF = mybir.ActivationFunctionType
EPS = 1e-5


def default_cfg():
    return dict(
        T=2048, D=256, N=8192, H=4, VOCAB=256, L=6, B=4,
        TCHUNK=512,          # tokens per chunk == attention query block
        mm_dt="f32r",        # "f32r" | "f32" : dtype view fed to the PE
        w_dt="bf16",         # "bf16" | "mm" : matmul dtype for the MLP path
        n_cores=8,
        reps=1,              # layer-stack repeats (for wall-clock timing deltas)
    )


def build_program(cfg):
    """Builds and compiles the per-core SPMD bass program."""
    T, D, VOCAB, L = cfg["T"], cfg["D"], cfg["VOCAB"], cfg["L"]
    NH = cfg["N"] // 2
    TC = cfg["TCHUNK"]
    TT = T // 128
    DT = D // 128
    VT = VOCAB // 128
    n_cores = cfg["n_cores"]
    assert D == 256 and TC % 128 == 0 and T % TC == 0 and T % 512 == 0

    MDT = mybir.dt.float32r if cfg["mm_dt"] == "f32r" else FP
    WDT = mybir.dt.bfloat16 if cfg.get("w_dt") == "bf16" else MDT

    nc = bacc.Bacc("TRN2", target_bir_lowering=False, debug=False,
                   num_devices=n_cores)

    idxf_d = nc.dram_tensor("idxf", [1, T], FP, kind="ExternalInput")
    wte_d = nc.dram_tensor("wte", [VT, 128, D], FP, kind="ExternalInput")
    dxh_d = nc.dram_tensor("dxh", [DT, 128, NH], WDT, kind="ExternalInput")
    dyh_d = nc.dram_tensor("dyh", [DT, 128, NH], WDT, kind="ExternalInput")
    eh_d = nc.dram_tensor("eh", [NH // 128, 128, D], WDT, kind="ExternalInput")
    ro_d = nc.dram_tensor("ro", [DT, 128, VOCAB], WDT, kind="ExternalInput")
    cosT_d = nc.dram_tensor("cosT", [DT, 128, T], WDT, kind="ExternalInput")
    sinT_d = nc.dram_tensor("sinT", [DT, 128, T], WDT, kind="ExternalInput")
    ident_d = nc.dram_tensor("ident", [128, 128], MDT, kind="ExternalInput")
    logits_d = nc.dram_tensor("logits", [TT, 128, VOCAB], FP,
                              kind="ExternalOutput")

    groups = [[2 * i, 2 * i + 1] for i in range(n_cores // 2)]

    with tile.TileContext(nc) as tc:
        with (
            tc.tile_pool(name="pers", bufs=1) as pers,
            tc.tile_pool(name="wk", bufs=3) as wk,
            tc.tile_pool(name="lat", bufs=2) as latp,
            tc.tile_pool(name="sm", bufs=4) as sm,
            tc.tile_pool(name="col", bufs=6) as col,
            tc.tile_pool(name="rp", bufs=2) as rp,
            tc.tile_pool(name="ps", bufs=4, space="PSUM") as ps,
            tc.tile_pool(name="acc", bufs=2, space="PSUM") as acc,
            tc.tile_pool(name="dram", bufs=1, space="DRAM") as dram,
        ):
            env = dict(nc=nc, cfg=cfg, MDT=MDT, WDT=WDT, wk=wk, sm=sm, col=col,
                       ps=ps, acc=acc, latp=latp, rp=rp, groups=groups)

            # ---------- persistent SBUF ----------
            ident = pers.tile([128, 128], MDT, tag="ident", name="ident")
            nc.sync.dma_start(ident[:], ident_d[:])
            env["ident"] = ident

            eps_col = pers.tile([128, 1], FP, tag="eps", name="eps_col")
            nc.vector.memset(eps_col[:], EPS)
            env["eps_col"] = eps_col

            wte = []
            for i in range(VT):
                w = pers.tile([128, D], FP, tag=f"wte{i}", name=f"wte{i}")
                nc.sync.dma_start(w[:], wte_d[i])
                wte.append(w)

            # rope tables first: embedding + layer-0 attention need them
            env["cosT"] = cosT = []
            env["sinT"] = sinT = []
            for i in range(DT):
                ct = pers.tile([128, T], WDT, tag=f"cosT{i}", name=f"cosT{i}")
                st = pers.tile([128, T], WDT, tag=f"sinT{i}", name=f"sinT{i}")
                nc.sync.dma_start(ct[:], cosT_d[i])
                nc.sync.dma_start(st[:], sinT_d[i])
                cosT.append(ct)
                sinT.append(st)

            env["dxh"] = dxh = []
            env["dyh"] = dyh = []
            for i in range(DT):
                dx = pers.tile([128, NH], WDT, tag=f"dxh{i}", name=f"dxh{i}")
                dy = pers.tile([128, NH], WDT, tag=f"dyh{i}", name=f"dyh{i}")
                nc.sync.dma_start(dx[:], dxh_d[i])
                nc.sync.dma_start(dy[:], dyh_d[i])
                dxh.append(dx)
                dyh.append(dy)

            env["ehs"] = ehs = []
            for m in range(NH // 128):
                e = pers.tile([128, D], WDT, tag=f"ehs{m}", name=f"ehs{m}")
                nc.sync.dma_start(e[:], eh_d[m])
                ehs.append(e)

            ro = []
            for i in range(DT):
                r = pers.tile([128, VOCAB], WDT, tag=f"ro{i}", name=f"ro{i}")
                nc.sync.dma_start(r[:], ro_d[i])
                ro.append(r)

            # double-buffered state: layer l reads set l%2, writes set (l+1)%2
            env["v_sb"] = [[pers.tile([128, D], MDT, tag=f"v{s}_{t}",
                                      name=f"v{s}_{t}")
                            for t in range(TT)] for s in range(2)]
            G = T // TC
            env["vT"] = [[[pers.tile([128, TC], WDT, tag=f"vT{s}_{i}_{g}",
                                     name=f"vT{s}_{i}_{g}")
                           for g in range(G)]
                          for i in range(DT)] for s in range(2)]
            env["qT"] = [[[pers.tile([128, TC], WDT, tag=f"qT{s}_{i}_{g}",
                                     name=f"qT{s}_{i}_{g}")
                           for g in range(G)]
                          for i in range(DT)] for s in range(2)]

            # ---------- embedding (writes set 0) ----------
            lnwte = []
            for i in range(VT):
                lw = pers.tile([128, D], MDT, tag=f"lnwte{i}", name=f"lnwte{i}")
                _ln_rows(env, lw, wte[i], D)
                lnwte.append(lw)

            ones1 = pers.tile([1, 128], FP, tag="ones1", name="ones1")
            nc.vector.memset(ones1[:], 1.0)
            iotav = []
            for i in range(VT):
                iv = pers.tile([128, 1], FP, tag=f"iotav{i}", name=f"iotav{i}")
                nc.gpsimd.iota(iv[:], pattern=[[0, 1]], base=i * 128,
                               channel_multiplier=1,
                               allow_small_or_imprecise_dtypes=True)
                iotav.append(iv)

            v_sb0, vT0 = env["v_sb"][0], env["vT"][0]
            # onehotT[v, t] = (idx[t] == v), built and consumed per 512-chunk
            with tc.tile_pool(name="emb", bufs=2) as embp:
                for c in range(T // 512):
                    cs = slice(c * 512, (c + 1) * 512)
                    idxf = embp.tile([1, 512], FP, tag="idxf", name="idxf")
                    nc.sync.dma_start(idxf[:], idxf_d[:, cs])
                    pidx = ps.tile([128, 512], FP, tag="mm", name="pidx")
                    nc.tensor.matmul(pidx[:], ones1[:], idxf[:],
                                     start=True, stop=True)
                    oh = []
                    for i in range(VT):
                        ohi = embp.tile([128, 512], MDT, tag="ohs", name="ohs")
                        nc.vector.tensor_scalar(ohi[:], pidx[:], iotav[i][:],
                                                None, op0=ALU.is_equal)
                        oh.append(ohi)
                    # v0 = LN(wte)[idx] for the 4 token tiles of this chunk
                    for tl in range(4):
                        t = c * 4 + tl
                        pv = ps.tile([128, D], FP, tag="mm", name="pv")
                        for i in range(VT):
                            nc.tensor.matmul(pv[:],
                                             (oh[i][:, tl * 128:(tl + 1) * 128]),
                                             (lnwte[i][:]),
                                             start=(i == 0), stop=(i == VT - 1))
                        nc.vector.tensor_copy(v_sb0[t][:], pv[:])
                    for i in range(DT):
                        pvt = ps.tile([128, 512], FP, tag="mm", name="pvt")
                        for k in range(VT):
                            nc.tensor.matmul(
                                pvt[:],
                                (lnwte[k][:, i * 128:(i + 1) * 128]),
                                (oh[k][:]),
                                start=(k == 0), stop=(k == VT - 1))
                        nc.vector.tensor_copy(vT0[i][c][:], pvt[:])

            env["up_dram"] = dram.tile([T, D], WDT, name="upd")
            env["upr_dram"] = dram.tile([T, D], WDT, name="uprd")

            # ---------- layers ----------
            total_layers = cfg["reps"] * L
            env["par"] = 0
            env["_pending_tail"] = None
            for li in range(total_layers):
                env["_layers_left"] = total_layers - 1 - li
                _emit_layer(env)
            assert env["_pending_tail"] is None

            # ---------- readout ----------
            vTF = env["vT"][env["par"]]
            NSUB = TC // 128
            for t in range(TT):
                tg, tc0 = t // NSUB, (t % NSUB) * 128
                pl = ps.tile([128, VOCAB], FP, tag="mm", name="pl")
                for i in range(DT):
                    nc.tensor.matmul(pl[:],
                                     (vTF[i][tg][:, tc0:tc0 + 128]),
                                     (ro[i][:]),
                                     start=(i == 0), stop=(i == DT - 1))
                lg = wk.tile([128, VOCAB], FP, tag="lg", name="lg")
                nc.vector.tensor_copy(lg[:], pl[:])
                nc.sync.dma_start(logits_d[t], lg[:])

    nc.compile()
    return nc


def _ln_rows(env, out_ap, in_ap, F, resid_ap=None):
    """LN over the free dim per partition row. If resid_ap: out = resid + ln(in)."""
    nc, sm, col = env["nc"], env["sm"], env["col"]
    st6 = col.tile([128, 6], FP, tag="bst", name="bst")
    nc.vector.bn_stats(st6[:], in_ap[:])
    st2 = col.tile([128, 2], FP, tag="bag", name="bag")
    nc.vector.bn_aggr(st2[:], st6[:])
    std = col.tile([128, 1], FP, tag="std", name="std")
    nc.scalar.activation(std[:], st2[:, 1:2], ACTF.Sqrt, bias=env["eps_col"][:])
    rstd = col.tile([128, 1], FP, tag="rstd", name="rstd")
    nc.vector.reciprocal(rstd[:], std[:])
    if resid_ap is None:
        nc.vector.tensor_scalar(out_ap[:], in_ap[:], st2[:, 0:1], rstd[:],
                                op0=ALU.subtract, op1=ALU.mult)
    else:
        tmp = sm.tile([128, F], FP, tag="lntmp", name="lntmp")
        nc.vector.tensor_scalar(tmp[:], in_ap[:], st2[:, 0:1], rstd[:],
                                op0=ALU.subtract, op1=ALU.mult)
        nc.vector.tensor_add(out_ap[:], resid_ap[:], tmp[:])


def _emit_rope(env, g, par):
    """qT[par][i][g] = rope(vT[par][.][g]) on DVE (bf16 2x mode)."""
    nc = env["nc"]
    cfg = env["cfg"]
    TC = cfg["TCHUNK"]
    DT = cfg["D"] // 128
    WDT = env["WDT"]
    rp = env["rp"]
    vT, qT = env["vT"][par], env["qT"][par]
    cosT, sinT = env["cosT"], env["sinT"]
    cs = slice(g * TC, (g + 1) * TC)
    for i in range(DT):
        o = 1 - i
        t1 = rp.tile([128, TC], WDT, tag="rope", bufs=4, name="ropeA")
        nc.vector.tensor_mul(t1[:], vT[i][g][:], cosT[i][:, cs])
        t2 = rp.tile([128, TC], WDT, tag="rope", bufs=4, name="ropeB")
        nc.vector.tensor_mul(t2[:], vT[o][g][:], sinT[i][:, cs])
        if i == 0:
            nc.vector.tensor_sub(qT[i][g][:], t1[:], t2[:])
        else:
            nc.vector.tensor_add(qT[i][g][:], t1[:], t2[:])


def _emit_layer(env):
    nc, cfg, MDT = env["nc"], env["cfg"], env["MDT"]
    WDT = env["WDT"]
    T, D = cfg["T"], cfg["D"]
    NH = cfg["N"] // 2
    TC = cfg["TCHUNK"]
    G = T // TC
    TT = T // 128
    MC = NH // 128
    NSUB = TC // 128
    DT = D // 128
    wk, sm, col, ps, acc = env["wk"], env["sm"], env["col"], env["ps"], env["acc"]
    latp = env["latp"]
    par = env["par"]
    v_sb, vT, qT = env["v_sb"][par], env["vT"][par], env["qT"][par]
    dxh, dyh, ehs = env["dxh"], env["dyh"], env["ehs"]
    ident = env["ident"]
    up_dram, upr_dram = env["up_dram"], env["upr_dram"]
    HALF = (G // 2) * TC                  # tokens per AllReduce half
    XPRE = 3                              # x m-chunks emitted ahead of y

    def emit_x(m, g, xrs):
        x_ps = ps.tile([128, TC], FP, tag="mm", name="x_ps")
        for i in range(DT):
            nc.tensor.matmul(x_ps[:],
                             dxh[i][:, m * 128:(m + 1) * 128],
                             vT[i][g][:],
                             start=(i == 0), stop=(i == DT - 1))
        xr = wk.tile([128, TC], WDT, tag="xr", bufs=12, name="xr")
        nc.scalar.activation(xr[:], x_ps[:], ACTF.Relu)
        xrs[m] = xr

    def emit_av(sb, off, e_sb, aT_ps, nkb):
        # aT[d, t] += v[s, d]^T e[s, t]  (free dim TC keeps f32r at full rate)
        # cols t < off of this block are zero by causality: skip them
        for i in range(DT):
            nc.tensor.matmul(aT_ps[:, i, off:],
                             v_sb[sb][:, i * 128:(i + 1) * 128],
                             e_sb[:, off:],
                             start=(sb == 0), stop=(sb == nkb - 1),
                             skip_group_check=True)

    def emit_ar(lo, hi):
        hs = slice(lo, hi)
        if cfg.get("no_cc"):
            nc.sync.dma_start(upr_dram[hs], up_dram[hs])
        else:
            nc.gpsimd.collective_compute(
                "AllReduce", ALU.add, replica_groups=env["groups"],
                ins=[up_dram[hs].opt()], outs=[upr_dram[hs].opt()])

    for g in range(G):
        t0 = g * TC
        nkb = (g + 1) * NSUB              # causal key-block count
        if g == 0 and not env.pop("_rope0_done", False):
            _emit_rope(env, 0, par)
        # ---- attention: energyT[s, t] blocks, AV lags two blocks (PE pipelining)
        aT_ps = acc.tile([128, DT, TC], FP, tag="acc", name="aT_ps")
        pend = []
        for sb in range(nkb):
            diag_j = max(0, sb - (nkb - NSUB))
            off = diag_j * 128        # cols t < off are masked anyway: skip
            e_ps = ps.tile([128, TC], FP, tag="mm", name="e_ps")
            sg, sc = sb // NSUB, (sb % NSUB) * 128
            for i in range(DT):
                nc.tensor.matmul(e_ps[:, off:],
                                 qT[i][sg][:, sc:sc + 128],
                                 qT[i][g][:, off:],
                                 start=(i == 0), stop=(i == DT - 1))
            e_sb = wk.tile([128, TC], MDT, tag="esb", bufs=5, name="e_sb")
            nc.scalar.copy(e_sb[:, off:], e_ps[:, off:])
            if sb - (nkb - NSUB) >= 0:
                # causal mask on the diagonal block: keep where t - s >= 0
                nc.gpsimd.affine_select(e_sb[:, off:], e_sb[:, off:],
                                        pattern=[[1, TC - off]],
                                        compare_op=ALU.is_ge, fill=0.0,
                                        base=0, channel_multiplier=-1)
            pend.append((sb, off, e_sb))
            if len(pend) > 3:
                s0, o0, e0 = pend.pop(0)
                emit_av(s0, o0, e0, aT_ps, nkb)
        # rope for the next chunk runs on DVE during this chunk's MLP
        if g + 1 < G:
            _emit_rope(env, g + 1, par)
        if env.get("_pending_evac") is not None:
            env["_pending_evac"]()
            env["_pending_evac"] = None
            if g == G // 2:
                emit_ar(0, HALF)     # both H0 chunks evacuated by now
        xrs = {}
        emit_x(0, g, xrs)
        for s0, o0, e0 in pend:
            emit_av(s0, o0, e0, aT_ps, nkb)
        pend = []
        emit_x(1, g, xrs)
        # ---- aT -> a (PE transposes), LN(a), la -> laT slices ----
        aT_sb = [wk.tile([128, TC], MDT, tag=f"aTs{i}", bufs=2, name="aT_sb")
                 for i in range(DT)]
        for i in range(DT):
            nc.scalar.copy(aT_sb[i][:], aT_ps[:, i, :])
        emit_x(2, g, xrs)
        laT = [latp.tile([128, TC], WDT, tag=f"laTs{i}", name=f"laT{i}")
               for i in range(DT)]
        def emit_ptr(tsub, la):
            for i in range(DT):
                ptr = ps.tile([128, 128], MDT, tag="mm", name="ptr")
                nc.tensor.matmul(ptr[:], la[:, i * 128:(i + 1) * 128], ident[:],
                                 is_transpose=True, start=True, stop=True)
                nc.scalar.copy(
                    laT[i][:, tsub * 128:(tsub + 1) * 128], ptr[:])

        nx = 3                       # next x m-chunk to prefetch
        emit_x(nx, g, xrs)
        nx += 1
        las = []
        for tsub in range(NSUB):
            a_ti = ps.tile([128, D], MDT, tag="mm", name="a_ti")
            for i in range(DT):
                nc.tensor.matmul(a_ti[:, i * 128:(i + 1) * 128],
                                 aT_sb[i][:, tsub * 128:(tsub + 1) * 128],
                                 ident[:], is_transpose=True,
                                 start=(i == 0), stop=(i == DT - 1),
                                 skip_group_check=True)
            la = sm.tile([128, D], MDT, tag="la", name="la")
            _ln_rows(env, la, a_ti, D)
            las.append(la)
            emit_x(nx, g, xrs)
            nx += 1
            if tsub < 2:             # extra cover while the first LN drains
                emit_x(nx, g, xrs)
                nx += 1
            if tsub >= 1:
                emit_ptr(tsub - 1, las[tsub - 1])
        emit_ptr(NSUB - 1, las[NSUB - 1])
        # ---- MLP: y = relu(Dy^T laT) * x, updateT += Eh^T yel, x pipelined ahead
        upT_ps = acc.tile([128, DT, TC], FP, tag="acc", name="upT_ps")

        def emit_up(m, yel):
            for i in range(DT):
                nc.tensor.matmul(upT_ps[:, i, :],
                                 ehs[m][:, i * 128:(i + 1) * 128],
                                 yel[:],
                                 start=(m == 0), stop=(m == MC - 1),
                                 skip_group_check=True)

        pups = []
        NPRE = 3 + NSUB + 3
        for m in range(MC):
            if m + NPRE < MC:
                emit_x(m + NPRE, g, xrs)
            y_ps = ps.tile([128, TC], FP, tag="mm", name="y_ps")
            for i in range(DT):
                nc.tensor.matmul(y_ps[:],
                                 dyh[i][:, m * 128:(m + 1) * 128],
                                 laT[i][:],
                                 start=(i == 0), stop=(i == DT - 1))
            # yel = relu(y) * relu(x) fused: (y_ps max 0) * xr in one DVE op
            yel = wk.tile([128, TC], WDT, tag="yel", bufs=4, name="yel")
            nc.vector.scalar_tensor_tensor(yel[:], y_ps[:], 0.0, xrs.pop(m)[:],
                                           op0=ALU.max, op1=ALU.mult)
            pups.append((m, yel))
            if len(pups) > 2:
                emit_up(*pups.pop(0))
        for pu in pups:
            emit_up(*pu)
        # ---- upT -> up evacuation, deferred into the next chunk's shadow ----
        def emit_upevac(t0=t0, upT_ps=upT_ps):
            upT_sb = [wk.tile([128, TC], MDT, tag=f"uTs{i}", bufs=1,
                              name="upT_sb")
                      for i in range(DT)]
            for i in range(DT):
                nc.scalar.copy(upT_sb[i][:], upT_ps[:, i, :])
            for tsub in range(NSUB):
                u_ti = ps.tile([128, D], MDT, tag="mm", name="u_ti")
                for i in range(DT):
                    nc.tensor.matmul(u_ti[:, i * 128:(i + 1) * 128],
                                     upT_sb[i][:, tsub * 128:(tsub + 1) * 128],
                                     ident[:], is_transpose=True,
                                     start=(i == 0), stop=(i == DT - 1),
                                     skip_group_check=True)
                u_sb = wk.tile([128, D], WDT, tag="usb", bufs=2, name="u_sb")
                nc.scalar.copy(u_sb[:], u_ti[:])
                r0 = t0 + tsub * 128
                nc.sync.dma_start(up_dram[r0:r0 + 128], u_sb[:])

        if g == G - 1:
            emit_upevac()
            emit_ar(HALF, T)
        else:
            env["_pending_evac"] = emit_upevac
        # ---- overlapped tails ----
        if g == 0 and env["_pending_tail"] is not None:
            # previous layer's second-half vnew, overlapping this chunk
            env["_pending_tail"]()
            env["_pending_tail"] = None
        if g == G - 2:
            # first-half vnew: AR(H0) had most of this chunk to complete
            _emit_vnew(env, 0, TT // 2, par)

    def tail(par=par):
        _emit_vnew(env, TT // 2, TT, par)

    if env["_layers_left"] > 0:
        env["_pending_tail"] = tail
        # next layer's chunk-0 rope: its vT (set 1-par) first half is ready
        _emit_rope(env, 0, 1 - par)
        env["_rope0_done"] = True
    else:
        tail()
    env["par"] = 1 - par


def _emit_vnew(env, t_lo, t_hi, par):
    """v[new][t] = v[par][t] + LN(upr[t]); refresh vT[new] via PE transposes."""
    nc, MDT = env["nc"], env["MDT"]
    D = env["cfg"]["D"]
    DT = D // 128
    sm, ps = env["sm"], env["ps"]
    TC = env["cfg"]["TCHUNK"]
    NSUB = TC // 128
    v_sb, vT = env["v_sb"][par], env["vT"][par]
    v_sbN, vTN = env["v_sb"][1 - par], env["vT"][1 - par]
    upr_dram = env["upr_dram"]
    for t in range(t_lo, t_hi):
        tg, tc0 = t // NSUB, (t % NSUB) * 128
        upr = sm.tile([128, D], env["WDT"], tag="upr", name="upr")
        nc.sync.dma_start(upr[:], upr_dram[t * 128:(t + 1) * 128])
        _ln_rows(env, v_sbN[t], upr, D, resid_ap=v_sb[t])
        for i in range(DT):
            ptr = ps.tile([128, 128], MDT, tag="mm", name="ptr2")
            nc.tensor.matmul(ptr[:], v_sbN[t][:, i * 128:(i + 1) * 128],
                             ident_of(env),
                             is_transpose=True, start=True, stop=True)
            nc.scalar.copy(vTN[i][tg][:, tc0:tc0 + 128], ptr[:])


def ident_of(env):
    return env["ident"][:]


# ====================== host side ======================

_BUILD_CACHE = {}


def shard_inputs(cfg, idx, wte, encoder, decoder_x, decoder_y, readout):
    """Returns the list of per-core input dicts."""
    import ml_dtypes
    wnp = ml_dtypes.bfloat16 if cfg.get("w_dt") == "bf16" else np.float32
    T, D, VOCAB = cfg["T"], cfg["D"], cfg["VOCAB"]
    NH = cfg["N"] // 2
    DT = D // 128
    VT = VOCAB // 128
    heads_per_half = cfg["H"] // 2

    inv_freq = 1.0 / (10000.0 ** (np.arange(0, D, 2, dtype=np.float64) / D))
    tpos = np.arange(T, dtype=np.float64)
    freqs = np.outer(tpos, inv_freq)
    emb = np.concatenate([freqs, freqs], axis=-1)     # [T, D]
    cosT = np.ascontiguousarray(np.cos(emb).T.astype(wnp)).reshape(DT, 128, T)
    sinT = np.ascontiguousarray(np.sin(emb).T.astype(wnp)).reshape(DT, 128, T)
    ident = np.eye(128, dtype=np.float32)

    wte_s = np.ascontiguousarray(wte.astype(np.float32)).reshape(VT, 128, D)
    ro_s = np.ascontiguousarray(readout.astype(wnp)).reshape(DT, 128, VOCAB)

    in_maps = []
    for c in range(cfg["n_cores"]):
        b, h = c // 2, c % 2
        heads = range(h * heads_per_half, (h + 1) * heads_per_half)
        dxh = np.concatenate([decoder_x[hh] for hh in heads], axis=-1)  # [D, NH]
        dyh = np.concatenate([decoder_y[hh] for hh in heads], axis=-1)
        ehh = encoder[h * NH:(h + 1) * NH]                              # [NH, D]
        in_maps.append(dict(
            idxf=np.ascontiguousarray(idx[b].astype(np.float32)[None, :]),
            wte=wte_s,
            dxh=np.ascontiguousarray(dxh.astype(wnp)).reshape(DT, 128, NH),
            dyh=np.ascontiguousarray(dyh.astype(wnp)).reshape(DT, 128, NH),
            eh=np.ascontiguousarray(ehh.astype(wnp)).reshape(NH // 128, 128, D),
            ro=ro_s,
            cosT=cosT, sinT=sinT, ident=ident,
        ))
    return in_maps


def get_program(cfg):
    key = (cfg["T"], cfg["N"], cfg["L"], cfg["mm_dt"], cfg.get("w_dt"),
           cfg["reps"], cfg["n_cores"], cfg["TCHUNK"], cfg.get("no_cc"))
    if key not in _BUILD_CACHE:
        _BUILD_CACHE[key] = build_program(cfg)
    return _BUILD_CACHE[key]


def kernel(idx, wte, encoder, decoder_x, decoder_y, readout):
    cfg = default_cfg()
    nc = get_program(cfg)
    in_maps = shard_inputs(cfg, np.asarray(idx), np.asarray(wte),
                           np.asarray(encoder), np.asarray(decoder_x),
                           np.asarray(decoder_y), np.asarray(readout))
    res = run_bass_kernel_spmd(nc, in_maps, list(range(cfg["n_cores"])))
    B, T, VOCAB = cfg["B"], cfg["T"], cfg["VOCAB"]
    out = np.empty((B, T, VOCAB), np.float32)
    for b in range(B):
        out[b] = res.results[2 * b]["logits"].reshape(T, VOCAB)
    return out
